# revision 4
# baseline (speedup 1.0000x reference)
"""Causal single-head attention (B=8, S=2048, E=768, H=64), v3: no P-transpose.

Data-parallel: one batch element per core. Per-core structure:
- proj QK (W-stationary, packed [Q*sqrt(E)|K]) -> qt_aug/kt_aug [65, S] fp16
  (row 64 of kt_aug = ones from host; row 64 of qt_aug = -rowmax, see below)
- proj V x-stationary directly into [k, h] layout v_sb [128, 16, 65] bf16
  with a ones column at h=64 (AV then yields softmax row-sums for free)
- pass-1: scores q-major in [128, 2, 512] psum pairs, DVE in-place diag
  mask add, DVE row maxes -> negm_all[:, t] (f32, negated)
- bias row per 4-tile group g: fp32 matmul negm[128,4].T @ I -> [4,128] in
  spare cols of the qk psum bank, DVE copy -> f16 SBUF, one SWDGE DMA
  reshapes [4,128] -> qt_aug[64, g*512:+512] (partition-major order match)
- pass-2: scores recomputed transposed WITH the max folded in: lhsT =
  kt_aug[:, j-block] [65,128] (row 64 = 1), rhs = qt_aug[:, cols] (row 64 =
  -m_q) -> s^T - m_q in psum; transposed mask added on diag blocks; exp
  (ACT, bias=0) writes P^T in BF16 straight to pt_all (AV lhsT layout, so
  no transpose of P anywhere). BF16 exponent range makes exp safe.
- AV tile i: sum_j pt_all[:, j, icols].T @ v_sb[:, j, :] -> o_g[:, 65-col
  group] psum; per-group drain: DVE reciprocal of col 64 + broadcast
  multiply, SWDGE store.

Emission interleaves pass-2/AV thunks of group g-1 between pass-1 tiles of
group g to keep the PE queue dense (HAM stays warm).
"""

import numpy as np
from contextlib import ExitStack

import concourse.bass as bass
import concourse.tile as tile
from concourse import bacc, mybir
from concourse.bass_utils import run_bass_kernel_spmd

F32 = mybir.dt.float32
F16 = mybir.dt.float16
BF16 = mybir.dt.bfloat16

B, S, E, H = 8, 2048, 768, 64
EC = E // 128          # 6 e-chunks
T = S // 128           # 16 query/key tiles
NEG = -1.0e9


def build_attention_core():
    nc = bacc.Bacc(None, target_bir_lowering=False)
    xt = nc.declare_dram_parameter("xt", (E, S), F16, isOutput=False)
    wqk = nc.declare_dram_parameter("wqk", (E, 128), F16, isOutput=False)
    wv = nc.declare_dram_parameter("wv", (E, H), F16, isOutput=False)
    mask = nc.declare_dram_parameter("mask", (128, 128), F32, isOutput=False)
    maskT = nc.declare_dram_parameter("maskT", (128, 128), F32, isOutput=False)
    ident = nc.declare_dram_parameter("ident", (128, 128), F32, isOutput=False)
    ones_row = nc.declare_dram_parameter("ones_row", (1, S), F16, isOutput=False)
    out = nc.declare_dram_parameter("out", (S, H), F32, isOutput=True)

    with ExitStack() as ctx:
        tc = ctx.enter_context(tile.TileContext(nc))
        singles = ctx.enter_context(tc.tile_pool(name="singles", bufs=1))
        # PSUM: oP 2 + s1P 2 + s2P 2 + qkP 1 + vP 1 = 8 banks
        oP = ctx.enter_context(tc.tile_pool(name="oP", bufs=2, space="PSUM"))
        s1P = ctx.enter_context(tc.tile_pool(name="s1P", bufs=1, space="PSUM"))
        s2P = ctx.enter_context(tc.tile_pool(name="s2P", bufs=1, space="PSUM"))
        stats = ctx.enter_context(tc.tile_pool(name="stats", bufs=6))
        nmt = ctx.enter_context(tc.tile_pool(name="nmt", bufs=2))
        ofin = ctx.enter_context(tc.tile_pool(name="ofin", bufs=2))

        # ---- loads: weights + xt block 0 on sync (HWDGE; this kernel has
        # no xbar transposes so plain HWDGE is safe), rest on SWDGE ----
        wqk_sb = singles.tile([128, EC, 128], F16)
        wv_sb = singles.tile([128, EC, H], F16)
        mask_sb = singles.tile([128, 128], F32)
        maskT_sb = singles.tile([128, 128], F32)
        ident_sb = singles.tile([128, 128], F32)
        qt_aug = singles.tile([65, S], F16)
        kt_aug = singles.tile([65, S], F16)
        xt_sb = singles.tile([128, EC, S], F16)

        nc.sync.dma_start(
            out=wqk_sb[:], in_=wqk.rearrange("(c p) m -> p c m", p=128))
        nc.sync.dma_start(
            out=wv_sb[:], in_=wv.rearrange("(c p) m -> p c m", p=128))
        for c in range(EC):
            nc.sync.dma_start(
                out=xt_sb[:, c, 0:512], in_=xt[c * 128:(c + 1) * 128, 0:512])
        nc.sync.dma_start(out=mask_sb[:], in_=mask[:])
        nc.sync.dma_start(out=maskT_sb[:], in_=maskT[:])
        nc.sync.dma_start(out=ident_sb[:], in_=ident[:])
        nc.sync.dma_start(out=kt_aug[64:65, :], in_=ones_row[:])
        for c in range(EC):
            nc.gpsimd.dma_start(
                out=xt_sb[:, c, 512:S], in_=xt[c * 128:(c + 1) * 128, 512:S])

        v_sb = singles.tile([128, T, H + 1], BF16)
        nc.vector.memset(v_sb[:, :, H:H + 1], 1.0)

        pt_all = singles.tile([128, T, S], BF16)
        negm_all = singles.tile([128, T], F32)

        o_tiles = {}

        # ---------------- emission helpers ----------------
        def emit_proj_qk(b, qkP):
            cols = bass.ts(b, 512)
            qk_ps = qkP.tile([128, 512], F32, tag="qk")
            for c in range(EC):
                nc.tensor.matmul(
                    qk_ps[:], lhsT=wqk_sb[:, c, :], rhs=xt_sb[:, c, cols],
                    start=(c == 0), stop=(c == EC - 1),
                )
            nc.scalar.copy(qt_aug[0:64, cols], qk_ps[0:64, :])
            nc.scalar.copy(kt_aug[0:64, cols], qk_ps[64:128, :])
            return qk_ps

        def emit_proj_v(b, vP):
            v_ps = vP.tile([128, 4, H], F32, tag="v")
            for jj in range(4):
                j = b * 4 + jj
                for c in range(EC):
                    nc.tensor.matmul(
                        v_ps[:, jj, :],
                        lhsT=xt_sb[:, c, j * 128:(j + 1) * 128],
                        rhs=wv_sb[:, c, :],
                        start=(c == 0), stop=(c == EC - 1),
                    )
            nc.vector.tensor_copy(v_sb[:, b * 4:(b + 1) * 4, 0:H], v_ps[:])

        def pass1_chunks(t):
            """q-major scores for tile t as a list of emission thunks (one
            per [128, 2, 512] psum pair) -> masked maxes -> negm_all[:, t]"""
            ki = (t + 1) * 128
            q_sl = bass.ts(t, 128)
            state = {}

            def mk(done, w, lastpair, bi0):
                def run():
                    if "mx" not in state:
                        state["mx"] = stats.tile([128, 4], F32, tag="mx", name="mx")
                    mx = state["mx"]
                    s_t = s1P.tile([128, 2, 512], F32, tag="s1")
                    nw = (w + 511) // 512
                    for h in range(nw):
                        ww = min(512, w - h * 512)
                        nc.tensor.matmul(
                            s_t[:, h, 0:ww],
                            lhsT=qt_aug[0:64, q_sl],
                            rhs=kt_aug[0:64, done + h * 512:done + h * 512 + ww],
                            start=True, stop=True,
                        )
                    if lastpair:
                        dh, dc = (w - 128) // 512, (w - 128) % 512
                        nc.vector.tensor_add(
                            s_t[:, dh, dc:dc + 128], s_t[:, dh, dc:dc + 128],
                            mask_sb[:],
                        )
                    for h in range(nw):
                        ww = min(512, w - h * 512)
                        nc.vector.tensor_reduce(
                            mx[:, bi0 + h:bi0 + h + 1], s_t[:, h, 0:ww],
                            axis=mybir.AxisListType.X, op=mybir.AluOpType.max,
                        )
                    if lastpair:
                        nc.vector.tensor_reduce(
                            negm_all[:, t:t + 1], mx[:, 0:bi0 + nw],
                            axis=mybir.AxisListType.X, op=mybir.AluOpType.max,
                            negate=True,
                        )
                return run

            chunks = []
            done = 0
            bi = 0
            while done < ki:
                w = min(1024, ki - done)
                chunks.append(mk(done, w, done + w == ki, bi))
                bi += (w + 511) // 512
                done += w
            return chunks

        def emit_bias_row(g, qk_ps):
            """negm_all[:, 4g:4g+4] -> qt_aug[64, g*512:(g+1)*512] (f16)"""
            tp = qk_ps[0:4, 384:512]
            nc.tensor.matmul(
                tp, lhsT=negm_all[:, 4 * g:4 * g + 4], rhs=ident_sb[:],
                start=True, stop=True,
            )
            nmt_sb = nmt.tile([4, 128], F16, tag="nmt")
            nc.vector.tensor_copy(nmt_sb[:], tp)
            qa = qt_aug[64:65, g * 512:(g + 1) * 512]
            out_ap = bass.AP(
                tensor=qa.tensor, offset=qa.offset,
                ap=[qa.ap[0], [128, 4], [1, 128]],
            )
            nc.gpsimd.dma_start(out=out_ap, in_=nmt_sb[:])

        def p2_slot(j, g, pool):
            """one pass-2 psum slot: transposed biased scores -> exp -> pt"""
            gc1 = (g + 1) * 512
            if j < 4 * g and j % 2 == 0:
                # off-diagonal pair (j, j+1) over the full 512 group cols
                s2 = pool.tile([128, 2, 512], F32, tag="s2")
                for h in range(2):
                    nc.tensor.matmul(
                        s2[:, h, :],
                        lhsT=kt_aug[:, (j + h) * 128:(j + h + 1) * 128],
                        rhs=qt_aug[:, g * 512:gc1],
                        start=True, stop=True,
                    )
                nc.scalar.activation(
                    pt_all[:, j:j + 2, g * 512:gc1], s2[:],
                    mybir.ActivationFunctionType.Exp,
                )
            else:
                # diagonal-group single, cols trimmed to q >= j*128
                c0 = j * 128
                w = gc1 - c0
                s2 = pool.tile([128, 2, 512], F32, tag="s2")
                nc.tensor.matmul(
                    s2[:, 0, 0:w],
                    lhsT=kt_aug[:, j * 128:(j + 1) * 128],
                    rhs=qt_aug[:, c0:gc1],
                    start=True, stop=True,
                )
                nc.vector.tensor_add(
                    s2[:, 0, 0:128], s2[:, 0, 0:128], maskT_sb[:])
                nc.scalar.activation(
                    pt_all[:, j, c0:gc1], s2[:, 0, 0:w],
                    mybir.ActivationFunctionType.Exp,
                )

        def emit_av(i):
            g = i // 4
            if g not in o_tiles:
                o_tiles[g] = oP.tile([128, 512], F32, tag="o", name="o_g")
            o_g = o_tiles[g]
            sl = (i % 4) * (H + 1)
            for j in range(i + 1):
                nc.tensor.matmul(
                    o_g[:, sl:sl + H + 1],
                    lhsT=pt_all[:, j, bass.ts(i, 128)],
                    rhs=v_sb[:, j, :],
                    start=(j == 0), stop=(j == i),
                )

        def p2_thunks(g, pools):
            """pass-2 + AV worklist for group g, as emission thunks"""
            th = []
            pi = [0]

            def mk(j):
                def run():
                    pool = pools[pi[0] % len(pools)]
                    pi[0] += 1
                    p2_slot(j, g, pool)
                    if j >= 4 * g:
                        emit_av(j)
                return run
            j = 0
            while j < 4 * g:
                th.append(mk(j))
                j += 2
            for j in range(4 * g, 4 * g + 4):
                th.append(mk(j))
            return th

        def emit_drain(g):
            """normalize + store group g"""
            o_ap = o_tiles[g][:]
            rs = stats.tile([128, 4], F32, tag="rs")
            sums_ap = bass.AP(
                tensor=o_ap.tensor, offset=o_ap.offset + H,
                ap=[o_ap.ap[0], [H + 1, 4], [0, 1]],
            )
            nc.vector.reciprocal(rs[:], sums_ap)
            of = ofin.tile([128, 4, H], F32, tag="of")
            o_data = bass.AP(
                tensor=o_ap.tensor, offset=o_ap.offset,
                ap=[o_ap.ap[0], [H + 1, 4], [1, H]],
            )
            rs_ap = rs[:]
            rs_b = bass.AP(
                tensor=rs_ap.tensor, offset=rs_ap.offset,
                ap=[rs_ap.ap[0], rs_ap.ap[1], [0, H]],
            )
            nc.vector.tensor_mul(of[:], o_data, rs_b)
            nc.gpsimd.dma_start(
                out=out.rearrange("(i p) h -> p i h", p=128)[:, 4 * g:4 * g + 4, :],
                in_=of[:],
            )

        # ---------------- schedule ----------------
        with (
            tc.tile_pool(name="qkP", bufs=1, space="PSUM") as qkP,
            tc.tile_pool(name="vP", bufs=1, space="PSUM") as vP,
        ):
            for b in range(4):
                qk_ps = emit_proj_qk(b, qkP)
                emit_proj_v(b, vP)
                # pass-1 pair-chunks of this group's 4 tiles, interleaved
                # round-robin with pass-2/AV thunks of group b-1
                p1w = []
                for t in range(4 * b, 4 * b + 4):
                    p1w.extend(pass1_chunks(t))
                p2w = p2_thunks(b - 1, [s2P]) if b >= 1 else []
                n1, n2 = len(p1w), len(p2w)
                wi = 0
                for k, c1 in enumerate(p1w):
                    c1()
                    tgt = (k + 1) * n2 // n1 if n1 else n2
                    # hold back a couple of thunks to cover the bias-row wait
                    while wi < min(tgt, max(n2 - 2, 0)):
                        p2w[wi]()
                        wi += 1
                emit_bias_row(b, qk_ps)
                while wi < n2:
                    p2w[wi]()
                    wi += 1
                if b >= 2:
                    emit_drain(b - 2)
        # tail: group 3 pass-2/AV with double-buffered psum slots
        with tc.tile_pool(name="s2Pb", bufs=1, space="PSUM") as s2Pb:
            for th in p2_thunks(3, [s2P, s2Pb]):
                th()
            emit_drain(2)
            emit_drain(3)

    nc.finalize()
    return nc


_NC_CACHE = None


def make_in_maps(x, Wq, Wk, Wv):
    scale = np.sqrt(np.float32(E))
    wqk_np = np.concatenate([(Wq * scale).T, Wk.T], axis=1).astype(np.float16)
    wv_np = Wv.T.astype(np.float16)
    mask_np = np.triu(np.full((128, 128), NEG, dtype=np.float32), k=1)
    maskT_np = np.ascontiguousarray(mask_np.T)
    ident_np = np.eye(128, dtype=np.float32)
    ones_np = np.ones((1, S), dtype=np.float16)
    return [
        {
            "xt": np.ascontiguousarray(x[b].T).astype(np.float16),
            "wqk": wqk_np,
            "wv": wv_np,
            "mask": mask_np,
            "maskT": maskT_np,
            "ident": ident_np,
            "ones_row": ones_np,
        }
        for b in range(B)
    ]


def kernel(x: np.ndarray, Wq: np.ndarray, Wk: np.ndarray, Wv: np.ndarray) -> np.ndarray:
    global _NC_CACHE
    assert x.shape == (B, S, E)
    in_maps = make_in_maps(x, Wq, Wk, Wv)
    if _NC_CACHE is None:
        _NC_CACHE = build_attention_core()
    res = run_bass_kernel_spmd(_NC_CACHE, in_maps, core_ids=list(range(B)))
    return np.stack([res.results[b]["out"] for b in range(B)], axis=0)


if __name__ == "__main__":
    d = np.load("/tmp/ref_cache.npz")
    o = kernel(x=d["x"], Wq=d["Wq"], Wk=d["Wk"], Wv=d["Wv"])
    exp = d["expected"]
    rel = np.linalg.norm(o - exp) / np.linalg.norm(exp)
    print("Relative error:", rel)


# revision 6
# speedup vs baseline: 1.3492x; 1.3492x over previous
"""Causal single-head attention (B=8, S=2048, E=768, H=64), v3: no P-transpose.

Data-parallel: one batch element per core. Per-core structure:
- proj QK (W-stationary, packed [Q*sqrt(E)|K]) -> qt_aug/kt_aug [65, S] fp16
  (row 64 of kt_aug = ones from host; row 64 of qt_aug = -rowmax, see below)
- proj V x-stationary directly into [k, h] layout v_sb [128, 16, 65] bf16
  with a ones column at h=64 (AV then yields softmax row-sums for free)
- pass-1: scores q-major in [128, 2, 512] psum pairs, DVE in-place diag
  mask add, DVE row maxes -> negm_all[:, t] (f32, negated)
- bias row per 4-tile group g: fp32 matmul negm[128,4].T @ I -> [4,128] in
  spare cols of the qk psum bank, DVE copy -> f16 SBUF, one SWDGE DMA
  reshapes [4,128] -> qt_aug[64, g*512:+512] (partition-major order match)
- pass-2: scores recomputed transposed WITH the max folded in: lhsT =
  kt_aug[:, j-block] [65,128] (row 64 = 1), rhs = qt_aug[:, cols] (row 64 =
  -m_q) -> s^T - m_q in psum; transposed mask added on diag blocks; exp
  (ACT, bias=0) writes P^T in BF16 straight to pt_all (AV lhsT layout, so
  no transpose of P anywhere). BF16 exponent range makes exp safe.
- AV tile i: sum_j pt_all[:, j, icols].T @ v_sb[:, j, :] -> o_g[:, 65-col
  group] psum; per-group drain: DVE reciprocal of col 64 + broadcast
  multiply, SWDGE store.

Emission interleaves pass-2/AV thunks of group g-1 between pass-1 tiles of
group g to keep the PE queue dense (HAM stays warm).
"""

import numpy as np
from contextlib import ExitStack

import concourse.bass as bass
import concourse.tile as tile
from concourse import bacc, mybir
from concourse.bass_utils import run_bass_kernel_spmd

F32 = mybir.dt.float32
F16 = mybir.dt.float16
BF16 = mybir.dt.bfloat16

B, S, E, H = 8, 2048, 768, 64
EC = E // 128          # 6 e-chunks
T = S // 128           # 16 query/key tiles
NEG = -1.0e9


def build_attention_core():
    nc = bacc.Bacc(None, target_bir_lowering=False)
    # xt packed per 512-col block: xtb[p, c, s] = x[b*512+s, c*128+p]
    xtbs = [
        nc.declare_dram_parameter(f"xt{b}", (128, EC, 512), F16, isOutput=False)
        for b in range(4)
    ]
    # wqkv[p, c, 0:128] = wqk chunk c; [:, c, 128:192] = wv chunk c
    wqkv = nc.declare_dram_parameter("wqkv", (128, EC, 192), F16, isOutput=False)
    # consts[:, 0, :]=mask, [:, 1, :]=maskT, [:, 2, :]=ident
    consts = nc.declare_dram_parameter("consts", (128, 3, 128), F32, isOutput=False)
    ones_row = nc.declare_dram_parameter("ones_row", (1, S), F16, isOutput=False)
    out = nc.declare_dram_parameter("out", (S, H), F32, isOutput=True)

    with ExitStack() as ctx:
        tc = ctx.enter_context(tile.TileContext(nc))
        singles = ctx.enter_context(tc.tile_pool(name="singles", bufs=1))
        # PSUM: oP 2 + s1P 2 + s2P 2 + qkP 1 + vP 1 = 8 banks
        oP = ctx.enter_context(tc.tile_pool(name="oP", bufs=1, space="PSUM"))
        s1P = ctx.enter_context(tc.tile_pool(name="s1P", bufs=4, space="PSUM"))
        s2P = ctx.enter_context(tc.tile_pool(name="s2P", bufs=3, space="PSUM"))
        stats = ctx.enter_context(tc.tile_pool(name="stats", bufs=6))
        nmt = ctx.enter_context(tc.tile_pool(name="nmt", bufs=2))
        ofin = ctx.enter_context(tc.tile_pool(name="ofin", bufs=2))

        # ---- loads: weights + xt block 0 on sync (HWDGE; this kernel has
        # no xbar transposes so plain HWDGE is safe), rest on SWDGE ----
        wqkv_sb = singles.tile([128, EC, 192], F16)
        consts_sb = singles.tile([128, 3, 128], F32)
        qt_aug = singles.tile([65, S], F16)
        kt_aug = singles.tile([65, S], F16)
        # xt as four per-block tiles so proj b waits only its own block
        xt_bs = [singles.tile([128, EC, 512], F16, name=f"xt_sb{b}", tag=f"xt{b}")
                 for b in range(4)]

        nc.sync.dma_start(out=wqkv_sb[:], in_=wqkv[:])
        nc.sync.dma_start(out=xt_bs[0][:], in_=xtbs[0][:])
        nc.sync.dma_start(out=consts_sb[:], in_=consts[:])
        for b in range(1, 4):
            nc.gpsimd.dma_start(out=xt_bs[b][:], in_=xtbs[b][:])
        nc.gpsimd.dma_start(out=kt_aug[64:65, :], in_=ones_row[:])

        wqk_sb = wqkv_sb[:, :, 0:128]
        wv_sb = wqkv_sb[:, :, 128:192]
        mask_sb = consts_sb[:, 0, :]
        maskT_sb = consts_sb[:, 1, :]
        ident_sb = consts_sb[:, 2, :]

        v_sb = singles.tile([128, T, H + 1], BF16)
        nc.vector.memset(v_sb[:, :, H:H + 1], 1.0)

        pt_all = singles.tile([128, T, S], BF16)
        negm_all = singles.tile([128, T], F32)

        o_tiles = {}

        # ---------------- emission helpers ----------------
        def emit_proj_qk(b):
            cols = bass.ts(b, 512)
            qk_ps = s1P.tile([128, 512], F32, tag="s1", name="qk_ps")
            for c in range(EC):
                nc.tensor.matmul(
                    qk_ps[:], lhsT=wqk_sb[:, c, :], rhs=xt_bs[b][:, c, :],
                    start=(c == 0), stop=(c == EC - 1),
                )
            nc.scalar.copy(qt_aug[0:64, cols], qk_ps[0:64, :])
            nc.scalar.copy(kt_aug[0:64, cols], qk_ps[64:128, :])
            return qk_ps

        def emit_proj_v(b):
            v_ps = s1P.tile([128, 512], F32, tag="s1", name="v_ps")
            for jj in range(4):
                for c in range(EC):
                    nc.tensor.matmul(
                        v_ps[:, jj * H:(jj + 1) * H],
                        lhsT=xt_bs[b][:, c, jj * 128:(jj + 1) * 128],
                        rhs=wv_sb[:, c, :],
                        start=(c == 0), stop=(c == EC - 1),
                    )
            vp = v_ps
            v_view = bass.AP(
                tensor=vp.tensor, offset=vp.offset,
                ap=[vp.ap[0], [H, 4], [1, H]],
            )
            nc.vector.tensor_copy(v_sb[:, b * 4:(b + 1) * 4, 0:H], v_view)

        def pass1_chunks(t):
            """q-major scores for tile t, one thunk per 512-col psum slot;
            masked maxes -> negm_all[:, t]"""
            ki = (t + 1) * 128
            nblk = (ki + 511) // 512
            q_sl = bass.ts(t, 128)
            state = {}

            def mk(bi, w, last):
                def run():
                    if "mx" not in state:
                        state["mx"] = stats.tile([128, 4], F32, tag="mx", name="mx")
                    mx = state["mx"]
                    s_t = s1P.tile([128, 512], F32, tag="s1")
                    nc.tensor.matmul(
                        s_t[:, 0:w],
                        lhsT=qt_aug[0:64, q_sl],
                        rhs=kt_aug[0:64, bi * 512:bi * 512 + w],
                        start=True, stop=True,
                    )
                    if last:
                        nc.vector.tensor_add(
                            s_t[:, w - 128:w], s_t[:, w - 128:w], mask_sb)
                    nc.vector.tensor_reduce(
                        mx[:, bi:bi + 1], s_t[:, 0:w],
                        axis=mybir.AxisListType.X, op=mybir.AluOpType.max,
                    )
                    if last:
                        nc.vector.tensor_reduce(
                            negm_all[:, t:t + 1], mx[:, 0:nblk],
                            axis=mybir.AxisListType.X, op=mybir.AluOpType.max,
                            negate=True,
                        )
                return run

            return [
                mk(bi, min(512, ki - bi * 512), bi == nblk - 1)
                for bi in range(nblk)
            ]

        def emit_bias_row(g):
            """negm_all[:, 4g:4g+4] -> qt_aug[64, g*512:(g+1)*512] (f16)"""
            tp_tile = s1P.tile([128, 512], F32, tag="s1", name="tp_tile")
            tp = tp_tile[0:4, 384:512]
            nc.tensor.matmul(
                tp, lhsT=negm_all[:, 4 * g:4 * g + 4], rhs=ident_sb,
                start=True, stop=True,
            )
            nmt_sb = nmt.tile([4, 128], F16, tag="nmt")
            nc.vector.tensor_copy(nmt_sb[:], tp)
            qa = qt_aug[64:65, g * 512:(g + 1) * 512]
            out_ap = bass.AP(
                tensor=qa.tensor, offset=qa.offset,
                ap=[qa.ap[0], [128, 4], [1, 128]],
            )
            nc.gpsimd.dma_start(out=out_ap, in_=nmt_sb[:])

        def p2_slot(j, g, pool, tag):
            """one pass-2 slot: transposed biased scores -> exp -> pt"""
            gc1 = (g + 1) * 512
            c0 = g * 512 if j < 4 * g else j * 128
            w = gc1 - c0
            s2 = pool.tile([128, 512], F32, tag=tag, name="s2t")
            nc.tensor.matmul(
                s2[:, 0:w],
                lhsT=kt_aug[:, j * 128:(j + 1) * 128],
                rhs=qt_aug[:, c0:gc1],
                start=True, stop=True,
            )
            if j >= 4 * g:
                nc.vector.tensor_add(s2[:, 0:128], s2[:, 0:128], maskT_sb)
            nc.scalar.activation(
                pt_all[:, j, c0:gc1], s2[:, 0:w],
                mybir.ActivationFunctionType.Exp,
            )

        def emit_av(i):
            g = i // 4
            if g not in o_tiles:
                o_tiles[g] = oP.tile([128, 512], F32, tag="o", name="o_g")
            o_g = o_tiles[g]
            sl = (i % 4) * (H + 1)
            for j in range(i + 1):
                nc.tensor.matmul(
                    o_g[:, sl:sl + H + 1],
                    lhsT=pt_all[:, j, bass.ts(i, 128)],
                    rhs=v_sb[:, j, :],
                    start=(j == 0), stop=(j == i),
                )

        pending_av = []  # (emit-after-slot-counter, tile index)
        slot_ctr = [0]

        def p2_thunks(g, pools):
            """pass-2 slots for group g + lagged AV conveyor thunks"""
            th = []
            pi = [0]

            def mk(j):
                def run():
                    pool, tag = pools[pi[0] % len(pools)]
                    pi[0] += 1
                    p2_slot(j, g, pool, tag)
                    slot_ctr[0] += 1
                    if j >= 4 * g:
                        pending_av.append((slot_ctr[0] + 2, j))
                    while pending_av and pending_av[0][0] <= slot_ctr[0]:
                        emit_av(pending_av.pop(0)[1])
                return run

            for j in range(4 * g + 4):
                th.append(mk(j))
            return th

        def flush_av():
            while pending_av:
                emit_av(pending_av.pop(0)[1])

        def emit_drain(g):
            """normalize + store group g"""
            o_ap = o_tiles[g][:]
            rs = stats.tile([128, 4], F32, tag="rs")
            sums_ap = bass.AP(
                tensor=o_ap.tensor, offset=o_ap.offset + H,
                ap=[o_ap.ap[0], [H + 1, 4], [0, 1]],
            )
            nc.vector.reciprocal(rs[:], sums_ap)
            of = ofin.tile([128, 4, H], F32, tag="of")
            o_data = bass.AP(
                tensor=o_ap.tensor, offset=o_ap.offset,
                ap=[o_ap.ap[0], [H + 1, 4], [1, H]],
            )
            rs_ap = rs[:]
            rs_b = bass.AP(
                tensor=rs_ap.tensor, offset=rs_ap.offset,
                ap=[rs_ap.ap[0], rs_ap.ap[1], [0, H]],
            )
            nc.vector.tensor_mul(of[:], o_data, rs_b)
            nc.gpsimd.dma_start(
                out=out.rearrange("(i p) h -> p i h", p=128)[:, 4 * g:4 * g + 4, :],
                in_=of[:],
            )

        # ---------------- schedule ----------------
        for b in range(4):
            emit_proj_qk(b)
            emit_proj_v(b)
            if b >= 2:
                flush_av()
                emit_drain(b - 2)
            p1w = []
            for t in range(4 * b, 4 * b + 4):
                p1w.extend(pass1_chunks(t))
            p2w = p2_thunks(b - 1, [(s2P, "s2")]) if b >= 1 else []
            n1, n2 = len(p1w), len(p2w)
            wi = 0
            for k, c1 in enumerate(p1w):
                c1()
                tgt = (k + 1) * n2 // n1 if n1 else n2
                while wi < min(tgt, max(n2 - 2, 0)):
                    p2w[wi]()
                    wi += 1
            emit_bias_row(b)
            while wi < n2:
                p2w[wi]()
                wi += 1
        # tail: group 3 pass-2/AV; reuse the idle s1 ring for extra slots
        flush_av()
        emit_drain(2)
        for th in p2_thunks(3, [(s2P, "s2"), (s1P, "s1")]):
            th()
        flush_av()
        emit_drain(3)

    nc.finalize()
    return nc


_NC_CACHE = None


def make_in_maps(x, Wq, Wk, Wv):
    scale = np.sqrt(np.float32(E))
    wqk_np = np.concatenate([(Wq * scale).T, Wk.T], axis=1).astype(np.float16)
    wv_np = Wv.T.astype(np.float16)
    # wqkv[p, c, 0:128] = wqk rows c*128..; [:, c, 128:192] = wv rows c*128..
    wqkv_np = np.concatenate(
        [wqk_np.reshape(EC, 128, 128), wv_np.reshape(EC, 128, H)], axis=2
    ).transpose(1, 0, 2).copy()
    mask_np = np.triu(np.full((128, 128), NEG, dtype=np.float32), k=1)
    consts_np = np.stack(
        [mask_np, np.ascontiguousarray(mask_np.T), np.eye(128, dtype=np.float32)],
        axis=1,
    ).astype(np.float32)  # [128, 3, 128]
    ones_np = np.ones((1, S), dtype=np.float16)
    maps = []
    for b in range(B):
        xt_b = np.ascontiguousarray(x[b].T).astype(np.float16)  # [E, S]
        m = {"wqkv": wqkv_np, "consts": consts_np, "ones_row": ones_np}
        for blk in range(4):
            # xtk[p, c, s] = x[blk*512+s, c*128+p] = xt_b[c*128+p, blk*512+s]
            m[f"xt{blk}"] = np.ascontiguousarray(
                xt_b.reshape(EC, 128, S)[:, :, blk * 512:(blk + 1) * 512]
                .transpose(1, 0, 2)
            )
        maps.append(m)
    return maps


def kernel(x: np.ndarray, Wq: np.ndarray, Wk: np.ndarray, Wv: np.ndarray) -> np.ndarray:
    global _NC_CACHE
    assert x.shape == (B, S, E)
    in_maps = make_in_maps(x, Wq, Wk, Wv)
    if _NC_CACHE is None:
        _NC_CACHE = build_attention_core()
    res = run_bass_kernel_spmd(_NC_CACHE, in_maps, core_ids=list(range(B)))
    return np.stack([res.results[b]["out"] for b in range(B)], axis=0)


if __name__ == "__main__":
    d = np.load("/tmp/ref_cache.npz")
    o = kernel(x=d["x"], Wq=d["Wq"], Wk=d["Wk"], Wv=d["Wv"])
    exp = d["expected"]
    rel = np.linalg.norm(o - exp) / np.linalg.norm(exp)
    print("Relative error:", rel)


# revision 8
# speedup vs baseline: 1.3843x; 1.0260x over previous
"""Causal single-head attention (B=8, S=2048, E=768, H=64), v3: no P-transpose.

Data-parallel: one batch element per core. Per-core structure:
- proj QK (W-stationary, packed [Q*sqrt(E)|K]) -> qt_aug/kt_aug [65, S] fp16
  (row 64 of kt_aug = ones from host; row 64 of qt_aug = -rowmax, see below)
- proj V x-stationary directly into [k, h] layout v_sb [128, 16, 65] bf16
  with a ones column at h=64 (AV then yields softmax row-sums for free)
- pass-1: scores q-major in [128, 2, 512] psum pairs, DVE in-place diag
  mask add, DVE row maxes -> negm_all[:, t] (f32, negated)
- bias row per 4-tile group g: fp32 matmul negm[128,4].T @ I -> [4,128] in
  spare cols of the qk psum bank, DVE copy -> f16 SBUF, one SWDGE DMA
  reshapes [4,128] -> qt_aug[64, g*512:+512] (partition-major order match)
- pass-2: scores recomputed transposed WITH the max folded in: lhsT =
  kt_aug[:, j-block] [65,128] (row 64 = 1), rhs = qt_aug[:, cols] (row 64 =
  -m_q) -> s^T - m_q in psum; transposed mask added on diag blocks; exp
  (ACT, bias=0) writes P^T in BF16 straight to pt_all (AV lhsT layout, so
  no transpose of P anywhere). BF16 exponent range makes exp safe.
- AV tile i: sum_j pt_all[:, j, icols].T @ v_sb[:, j, :] -> o_g[:, 65-col
  group] psum; per-group drain: DVE reciprocal of col 64 + broadcast
  multiply, SWDGE store.

Emission interleaves pass-2/AV thunks of group g-1 between pass-1 tiles of
group g to keep the PE queue dense (HAM stays warm).
"""

import numpy as np
from contextlib import ExitStack

import concourse.bass as bass
import concourse.tile as tile
from concourse import bacc, mybir
from concourse.bass_utils import run_bass_kernel_spmd

F32 = mybir.dt.float32
F16 = mybir.dt.float16
BF16 = mybir.dt.bfloat16

B, S, E, H = 8, 2048, 768, 64
EC = E // 128          # 6 e-chunks
T = S // 128           # 16 query/key tiles
NEG = -1.0e9


def build_attention_core():
    nc = bacc.Bacc(None, target_bir_lowering=False)
    # xt packed per 512-col block: xtb[p, c, s] = x[b*512+s, c*128+p]
    xtbs = [
        nc.declare_dram_parameter(f"xt{b}", (128, EC, 512), F16, isOutput=False)
        for b in range(4)
    ]
    # wqkv[p, c, 0:128] = wqk chunk c; [:, c, 128:192] = wv chunk c
    wqkv = nc.declare_dram_parameter("wqkv", (128, EC, 192), F16, isOutput=False)
    # consts[:, 0, :]=mask, [:, 1, :]=maskT, [:, 2, :]=ident
    consts = nc.declare_dram_parameter("consts", (128, 3, 128), F32, isOutput=False)
    ones_row = nc.declare_dram_parameter("ones_row", (1, S), F16, isOutput=False)
    out = nc.declare_dram_parameter("out", (S, H), F32, isOutput=True)

    with ExitStack() as ctx:
        tc = ctx.enter_context(tile.TileContext(nc))
        singles = ctx.enter_context(tc.tile_pool(name="singles", bufs=1))
        # PSUM: oP 2 + s1P 2 + s2P 2 + qkP 1 + vP 1 = 8 banks
        oP = ctx.enter_context(tc.tile_pool(name="oP", bufs=1, space="PSUM"))
        s1P = ctx.enter_context(tc.tile_pool(name="s1P", bufs=4, space="PSUM"))
        s2P = ctx.enter_context(tc.tile_pool(name="s2P", bufs=3, space="PSUM"))
        stats = ctx.enter_context(tc.tile_pool(name="stats", bufs=6))
        nmt = ctx.enter_context(tc.tile_pool(name="nmt", bufs=2))
        ofin = ctx.enter_context(tc.tile_pool(name="ofin", bufs=2))

        # ---- loads: weights + xt block 0 on sync (HWDGE; this kernel has
        # no xbar transposes so plain HWDGE is safe), rest on SWDGE ----
        wqkv_sb = singles.tile([128, EC, 192], F16)
        consts_sb = singles.tile([128, 3, 128], F32)
        qt_aug = singles.tile([65, S], F16)
        kt_aug = singles.tile([65, S], F16)
        # xt as four per-block tiles so proj b waits only its own block
        xt_bs = [singles.tile([128, EC, 512], F16, name=f"xt_sb{b}", tag=f"xt{b}")
                 for b in range(4)]

        nc.sync.dma_start(out=wqkv_sb[:], in_=wqkv[:])
        nc.sync.dma_start(out=xt_bs[0][:], in_=xtbs[0][:])
        nc.sync.dma_start(out=consts_sb[:], in_=consts[:])
        for b in range(1, 4):
            nc.gpsimd.dma_start(out=xt_bs[b][:], in_=xtbs[b][:])
        nc.gpsimd.dma_start(out=kt_aug[64:65, :], in_=ones_row[:])

        wqk_sb = wqkv_sb[:, :, 0:128]
        wv_sb = wqkv_sb[:, :, 128:192]
        mask_sb = consts_sb[:, 0, :]
        maskT_sb = consts_sb[:, 1, :]
        ident_sb = consts_sb[:, 2, :]

        v_sb = singles.tile([128, T, H + 1], BF16)
        nc.vector.memset(v_sb[:, :, H:H + 1], 1.0)

        # PE warmup: ~3.5us of tiny back-to-back matmuls while loads are in
        # flight, so the HAM un-throttles the PE clock before real work
        warm_sb = singles.tile([16, 32], F16)
        nc.vector.memset(warm_sb[:], 0.25)
        wm_ps = s1P.tile([128, 512], F32, tag="s1", name="wm_ps")
        for i in range(64):
            nc.tensor.matmul(
                wm_ps[0:32, 0:32], lhsT=warm_sb[:, 0:32], rhs=warm_sb[:, 0:32],
                start=True, stop=True,
            )
        wjunk = stats.tile([128, 1], F32, tag="wjunk")
        nc.vector.tensor_reduce(
            wjunk[0:32, :], wm_ps[0:32, 0:32],
            axis=mybir.AxisListType.X, op=mybir.AluOpType.max,
        )

        pt_all = singles.tile([128, T, S], BF16)
        negm_all = singles.tile([128, T], F32)

        o_tiles = {}

        # ---------------- emission helpers ----------------
        def emit_proj_qk(b):
            cols = bass.ts(b, 512)
            qk_ps = s1P.tile([128, 512], F32, tag="s1", name="qk_ps")
            for c in range(EC):
                nc.tensor.matmul(
                    qk_ps[:], lhsT=wqk_sb[:, c, :], rhs=xt_bs[b][:, c, :],
                    start=(c == 0), stop=(c == EC - 1),
                )
            nc.scalar.copy(qt_aug[0:64, cols], qk_ps[0:64, :])
            nc.scalar.copy(kt_aug[0:64, cols], qk_ps[64:128, :])
            return qk_ps

        def emit_proj_v(b):
            v_ps = s1P.tile([128, 512], F32, tag="s1", name="v_ps")
            for jj in range(4):
                for c in range(EC):
                    nc.tensor.matmul(
                        v_ps[:, jj * H:(jj + 1) * H],
                        lhsT=xt_bs[b][:, c, jj * 128:(jj + 1) * 128],
                        rhs=wv_sb[:, c, :],
                        start=(c == 0), stop=(c == EC - 1),
                    )
            vp = v_ps
            v_view = bass.AP(
                tensor=vp.tensor, offset=vp.offset,
                ap=[vp.ap[0], [H, 4], [1, H]],
            )
            nc.vector.tensor_copy(v_sb[:, b * 4:(b + 1) * 4, 0:H], v_view)

        def pass1_chunks(t):
            """q-major scores for tile t, one thunk per 512-col psum slot;
            masked maxes -> negm_all[:, t]"""
            ki = (t + 1) * 128
            nblk = (ki + 511) // 512
            q_sl = bass.ts(t, 128)
            state = {}

            def mk(bi, w, last):
                def run():
                    if "mx" not in state:
                        state["mx"] = stats.tile([128, 4], F32, tag="mx", name="mx")
                    mx = state["mx"]
                    s_t = s1P.tile([128, 512], F32, tag="s1")
                    nc.tensor.matmul(
                        s_t[:, 0:w],
                        lhsT=qt_aug[0:64, q_sl],
                        rhs=kt_aug[0:64, bi * 512:bi * 512 + w],
                        start=True, stop=True,
                    )
                    if last:
                        nc.vector.tensor_add(
                            s_t[:, w - 128:w], s_t[:, w - 128:w], mask_sb)
                    nc.vector.tensor_reduce(
                        mx[:, bi:bi + 1], s_t[:, 0:w],
                        axis=mybir.AxisListType.X, op=mybir.AluOpType.max,
                    )
                    if last:
                        nc.vector.tensor_reduce(
                            negm_all[:, t:t + 1], mx[:, 0:nblk],
                            axis=mybir.AxisListType.X, op=mybir.AluOpType.max,
                            negate=True,
                        )
                return run

            return [
                mk(bi, min(512, ki - bi * 512), bi == nblk - 1)
                for bi in range(nblk)
            ]

        def emit_bias_row(g):
            """negm_all[:, 4g:4g+4] -> qt_aug[64, g*512:(g+1)*512] (f16)"""
            tp_tile = s1P.tile([128, 512], F32, tag="s1", name="tp_tile")
            tp = tp_tile[0:4, 384:512]
            nc.tensor.matmul(
                tp, lhsT=negm_all[:, 4 * g:4 * g + 4], rhs=ident_sb,
                start=True, stop=True,
            )
            nmt_sb = nmt.tile([4, 128], F16, tag="nmt")
            nc.vector.tensor_copy(nmt_sb[:], tp)
            qa = qt_aug[64:65, g * 512:(g + 1) * 512]
            out_ap = bass.AP(
                tensor=qa.tensor, offset=qa.offset,
                ap=[qa.ap[0], [128, 4], [1, 128]],
            )
            nc.gpsimd.dma_start(out=out_ap, in_=nmt_sb[:])

        def p2_slot(j, g, pool, tag):
            """one pass-2 slot: transposed biased scores -> exp -> pt"""
            gc1 = (g + 1) * 512
            c0 = g * 512 if j < 4 * g else j * 128
            w = gc1 - c0
            s2 = pool.tile([128, 512], F32, tag=tag, name="s2t")
            nc.tensor.matmul(
                s2[:, 0:w],
                lhsT=kt_aug[:, j * 128:(j + 1) * 128],
                rhs=qt_aug[:, c0:gc1],
                start=True, stop=True,
            )
            if j >= 4 * g:
                nc.vector.tensor_add(s2[:, 0:128], s2[:, 0:128], maskT_sb)
            nc.scalar.activation(
                pt_all[:, j, c0:gc1], s2[:, 0:w],
                mybir.ActivationFunctionType.Exp,
            )

        def emit_av(i):
            g = i // 4
            if g not in o_tiles:
                o_tiles[g] = oP.tile([128, 512], F32, tag="o", name="o_g")
            o_g = o_tiles[g]
            sl = (i % 4) * (H + 1)
            for j in range(i + 1):
                nc.tensor.matmul(
                    o_g[:, sl:sl + H + 1],
                    lhsT=pt_all[:, j, bass.ts(i, 128)],
                    rhs=v_sb[:, j, :],
                    start=(j == 0), stop=(j == i),
                )

        pending_av = []  # (emit-after-slot-counter, tile index)
        slot_ctr = [0]

        def p2_thunks(g, pools):
            """pass-2 slots for group g + lagged AV conveyor thunks"""
            th = []
            pi = [0]

            def mk(j):
                def run():
                    pool, tag = pools[pi[0] % len(pools)]
                    pi[0] += 1
                    p2_slot(j, g, pool, tag)
                    slot_ctr[0] += 1
                    if j >= 4 * g:
                        pending_av.append((slot_ctr[0] + 3, j))
                    while pending_av and pending_av[0][0] <= slot_ctr[0]:
                        emit_av(pending_av.pop(0)[1])
                return run

            for j in range(4 * g + 4):
                th.append(mk(j))
            return th

        def flush_av():
            while pending_av:
                emit_av(pending_av.pop(0)[1])

        def emit_drain(g):
            """normalize + store group g"""
            o_ap = o_tiles[g][:]
            rs = stats.tile([128, 4], F32, tag="rs")
            sums_ap = bass.AP(
                tensor=o_ap.tensor, offset=o_ap.offset + H,
                ap=[o_ap.ap[0], [H + 1, 4], [0, 1]],
            )
            nc.vector.reciprocal(rs[:], sums_ap)
            of = ofin.tile([128, 4, H], F32, tag="of")
            o_data = bass.AP(
                tensor=o_ap.tensor, offset=o_ap.offset,
                ap=[o_ap.ap[0], [H + 1, 4], [1, H]],
            )
            rs_ap = rs[:]
            rs_b = bass.AP(
                tensor=rs_ap.tensor, offset=rs_ap.offset,
                ap=[rs_ap.ap[0], rs_ap.ap[1], [0, H]],
            )
            nc.vector.tensor_mul(of[:], o_data, rs_b)
            nc.gpsimd.dma_start(
                out=out.rearrange("(i p) h -> p i h", p=128)[:, 4 * g:4 * g + 4, :],
                in_=of[:],
            )

        # ---------------- schedule ----------------
        for b in range(4):
            emit_proj_qk(b)
            emit_proj_v(b)
            if b >= 2:
                flush_av()
                emit_drain(b - 2)
            p1w = []
            for t in range(4 * b, 4 * b + 4):
                p1w.extend(pass1_chunks(t))
            p2w = p2_thunks(
                b - 1, [(s2P, "s2"), (s2P, "s2"), (s1P, "s1")]) if b >= 1 else []
            n1, n2 = len(p1w), len(p2w)
            wi = 0
            for k, c1 in enumerate(p1w):
                c1()
                tgt = (k + 1) * n2 // n1 if n1 else n2
                while wi < min(tgt, max(n2 - 2, 0)):
                    p2w[wi]()
                    wi += 1
            emit_bias_row(b)
            while wi < n2:
                p2w[wi]()
                wi += 1
        # tail: group 3 pass-2/AV; reuse the idle s1 ring for extra slots
        flush_av()
        emit_drain(2)
        for th in p2_thunks(3, [(s2P, "s2"), (s1P, "s1")]):
            th()
        flush_av()
        emit_drain(3)

    nc.finalize()
    return nc


_NC_CACHE = None


def make_in_maps(x, Wq, Wk, Wv):
    scale = np.sqrt(np.float32(E))
    wqk_np = np.concatenate([(Wq * scale).T, Wk.T], axis=1).astype(np.float16)
    wv_np = Wv.T.astype(np.float16)
    # wqkv[p, c, 0:128] = wqk rows c*128..; [:, c, 128:192] = wv rows c*128..
    wqkv_np = np.concatenate(
        [wqk_np.reshape(EC, 128, 128), wv_np.reshape(EC, 128, H)], axis=2
    ).transpose(1, 0, 2).copy()
    mask_np = np.triu(np.full((128, 128), NEG, dtype=np.float32), k=1)
    consts_np = np.stack(
        [mask_np, np.ascontiguousarray(mask_np.T), np.eye(128, dtype=np.float32)],
        axis=1,
    ).astype(np.float32)  # [128, 3, 128]
    ones_np = np.ones((1, S), dtype=np.float16)
    maps = []
    for b in range(B):
        xt_b = np.ascontiguousarray(x[b].T).astype(np.float16)  # [E, S]
        m = {"wqkv": wqkv_np, "consts": consts_np, "ones_row": ones_np}
        for blk in range(4):
            # xtk[p, c, s] = x[blk*512+s, c*128+p] = xt_b[c*128+p, blk*512+s]
            m[f"xt{blk}"] = np.ascontiguousarray(
                xt_b.reshape(EC, 128, S)[:, :, blk * 512:(blk + 1) * 512]
                .transpose(1, 0, 2)
            )
        maps.append(m)
    return maps


def kernel(x: np.ndarray, Wq: np.ndarray, Wk: np.ndarray, Wv: np.ndarray) -> np.ndarray:
    global _NC_CACHE
    assert x.shape == (B, S, E)
    in_maps = make_in_maps(x, Wq, Wk, Wv)
    if _NC_CACHE is None:
        _NC_CACHE = build_attention_core()
    res = run_bass_kernel_spmd(_NC_CACHE, in_maps, core_ids=list(range(B)))
    return np.stack([res.results[b]["out"] for b in range(B)], axis=0)


if __name__ == "__main__":
    d = np.load("/tmp/ref_cache.npz")
    o = kernel(x=d["x"], Wq=d["Wq"], Wk=d["Wk"], Wv=d["Wv"])
    exp = d["expected"]
    rel = np.linalg.norm(o - exp) / np.linalg.norm(exp)
    print("Relative error:", rel)
